# revision 7
# baseline (speedup 1.0000x reference)
"""Windowed correlation (cost volume) kernel for Trainium2, 8 NeuronCores.

Problem: feature1, feature2 (8, 128, 128, 256) fp32 -> out (8, 81, 128, 256),
out[b, ki*9+kj, y, x] = (1/128) * sum_c f1[b,c,y,x] * f2pad[b,c,y+ki,x+kj].

Strategy (v2):
  - Data-parallel over batch: core i handles batch i (c=128 on the SBUF
    partitions; contraction over c on the TensorEngine).
  - Host marshals: f1 im2col-packed per (8y x 16x) pixel block, pre-scaled by
    1/128, bf16; f2 zero-padded (halo 4) bf16.
  - Per pixel block one bf16 matmul: lhsT = f1 block [c, 128pix], rhs = the
    16x24=384-col padded f2 halo block [c, 384].  The 81 useful products per
    pixel sit at cols (ry+ki)*24 + rx+kj, i.e. at a per-PIXEL base
    b(ry,rx) = 24*ry + rx plus partition-UNIFORM offsets 24*ki + kj.
  - Shear resolution (the key change vs v1): the full Gram tiles round-trip
    through a DRAM scratch laid out [x0][pix][col] so that for each output
    partition q=(ry,xc) ALL data needed by its 16 pixels is ONE contiguous
    5976-elem run starting at the affine address xc*49152 + ry*6168.  A single
    3-dim-AP DMA per row (128 x 12KB descriptors) gathers those runs back into
    SBUF; an ACT strided copy [xm*385 + 24*ki + kj] then compacts the bands.
    This replaces v1's 2304 tiny (18B-descriptor) DMAs, which saturated both
    the issuing queues (~650ns/DMA) and the DMA engines (~35ns/18B descriptor).
  - Device writes out[y, x, d] bf16; host transposes to [d, y, x] fp32.

Engine plan per row y0 (pipelined):
  ACT    f2row im2col copy (y0); band-select of y0-1; both 4-dim-AP copies
  PE     16 matmuls (y0) into 4 rotating PSUM banks
  DVE    16 psum->stage bf16 copies (y0)
  SP     f2p load; per-row f1 loads; stage->scr store; ob->out store
  POOL   shear-gather scr->gb (y0-1)
"""

import numpy as np

_B, _C, _H, _W = 8, 128, 128, 256
_K = 9            # kernel size (2*max_disp+1)
_ND = _K * _K     # 81 displacements
_BY, _BX = 8, 16  # pixel block (M = 128 = PE rows)
_NBY, _NBX = _H // _BY, _W // _BX        # 16 x 16 blocks
_NA, _NB = _BY + _K - 1, _BX + _K - 1    # 16 x 24 halo block
_NCOLS = _NA * _NB                       # 384 psum columns
_HP, _WP = _H + _K - 1, _W + _K - 1      # padded f2 dims
_NPS = 4                                 # rotating psum banks

_BLK = 128 * _NCOLS                      # 49152: scr stride per x0 block
_RYSTR = _BX * _NCOLS + _NB              # 6168: scr stride per ry (shear)
_RUN = (_BX - 1) * (_NCOLS + 1) + (_K - 1) * _NB + _K  # 5976
_SLAB = _NBX * _BLK + 512                # padded row slab (786944)
_OUTROW = _BY * _W * _ND                 # 165888 out elems per row slab

_CACHE = {}


def _build_nc():
    from contextlib import ExitStack

    import concourse.bass as bass
    import concourse.mybir as mybir

    nc = bass.Bass()
    # f1 host-packed [c, y0, x0*128 + ry*16 + rx], pre-scaled by 1/c, bf16
    f1 = nc.dram_tensor(
        "f1", [_C, _NBY, _NBX * 128], mybir.dt.bfloat16, kind="ExternalInput"
    )
    # f2 zero-padded [c, 136, 264] bf16
    f2 = nc.dram_tensor("f2", [_C, _HP, _WP], mybir.dt.bfloat16, kind="ExternalInput")
    # out[y0][ (ry*256 + x)*81 + d ] bf16; host reshapes/transposes
    out = nc.dram_tensor("out", [_NBY, _OUTROW], mybir.dt.bfloat16, kind="ExternalOutput")
    # scr row slab layout: [x0][pix][col] (addr = x0*49152 + pix*384 + col)
    scr = nc.dram_tensor("scr", [_NBY, _SLAB], mybir.dt.bfloat16, kind="Internal")

    with ExitStack() as ctx:
        f1r = [
            ctx.enter_context(nc.sbuf_tensor(f"f1r{i}", [_C, _NBX * 128], mybir.dt.bfloat16))
            for i in range(2)
        ]
        f2p = ctx.enter_context(nc.sbuf_tensor("f2p", [_C, _HP * _WP], mybir.dt.bfloat16))
        f2row = [
            ctx.enter_context(nc.sbuf_tensor(f"f2r{i}", [_C, _NBX * _NCOLS], mybir.dt.bfloat16))
            for i in range(2)
        ]
        stage = [
            ctx.enter_context(nc.sbuf_tensor(f"stg{i}", [128, _NBX * _NCOLS], mybir.dt.bfloat16))
            for i in range(2)
        ]
        gb = [
            ctx.enter_context(nc.sbuf_tensor(f"gb{i}", [128, _RUN], mybir.dt.bfloat16))
            for i in range(2)
        ]
        ob = [
            ctx.enter_context(nc.sbuf_tensor(f"ob{i}", [128, _NBX * _ND], mybir.dt.bfloat16))
            for i in range(2)
        ]
        psum = [
            ctx.enter_context(nc.psum_tensor(f"ps{i}", [128, _NCOLS], mybir.dt.float32))
            for i in range(_NPS)
        ]
        # DMA-completion semaphores are per double-buffer parity: completions of
        # DMAs issued on the same queue are NOT ordered, so a cumulative count
        # on one semaphore could be satisfied by a later DMA finishing first.
        s_f1 = [ctx.enter_context(nc.semaphore(name=f"s_f1{i}")) for i in range(2)]
        s_f2 = ctx.enter_context(nc.semaphore(name="s_f2"))    # +16 (f2p load)
        s_act = ctx.enter_context(nc.semaphore(name="s_act"))  # +1 per f2row im2col
        s_pe = ctx.enter_context(nc.semaphore(name="s_pe"))    # +1 per matmul
        s_dve = ctx.enter_context(nc.semaphore(name="s_dve"))  # +1 per stage copy
        s_st = [ctx.enter_context(nc.semaphore(name=f"s_st{i}")) for i in range(2)]
        s_g = [ctx.enter_context(nc.semaphore(name=f"s_g{i}")) for i in range(2)]
        s_sel = ctx.enter_context(nc.semaphore(name="s_sel"))  # +1 per band select
        s_out = [ctx.enter_context(nc.semaphore(name=f"s_out{i}")) for i in range(2)]
        blk = ctx.enter_context(nc.Block())

        def _load_f1(sync, r):
            if r >= 2:  # WAR: matmuls of r-2 read f1r[r%2]
                sync.wait_ge(s_pe, (r - 1) * _NBX)
            src = bass.AP(
                tensor=f1,
                offset=r * _NBX * 128,
                ap=[[_NBY * _NBX * 128, _C], [1, _NBX * 128]],
            )
            sync.dma_start(f1r[r % 2][:, :], src).then_inc(s_f1[r % 2], 16)

        @blk.sync
        def _(sync):
            def _gathers(r):
                # shear-gather row r (HWDGE: SWDGE pays ~800ns/descriptor extra).
                # store(r) completed during the previous row's epoch -> no stall.
                sync.wait_ge(s_st[r % 2], (r // 2 + 1) * 16)
                if r >= 2:  # WAR: select of r-2 read gb[r%2]
                    sync.wait_ge(s_sel, r - 1)
                for ry in range(_BY):
                    src = bass.AP(
                        tensor=scr,
                        offset=r * _SLAB + ry * _RYSTR,
                        ap=[[_BLK, _NBX], [1, _RUN]],
                    )
                    dst = bass.AP(
                        tensor=gb[r % 2],
                        offset=ry * _NBX * _RUN,
                        ap=[[_RUN, _NBX], [1, _RUN]],
                    )
                    sync.dma_start(dst, src).then_inc(s_g[r % 2], 16)

            def _store_out(r):
                sync.wait_ge(s_sel, r + 1)
                dst2 = bass.AP(
                    tensor=out,
                    offset=r * _OUTROW,
                    ap=[[_NBX * _ND, 128], [1, _NBX * _ND]],
                )
                sync.dma_start(dst2, ob[r % 2][:, :]).then_inc(s_out[r % 2], 16)

            _load_f1(sync, 0)
            sync.dma_start(f2p[:, :], f2.ap().rearrange("c h w -> c (h w)")).then_inc(
                s_f2, 16
            )
            _load_f1(sync, 1)
            for r in range(_NBY):
                # store row r's Gram tiles once its 16 stage copies are done
                sync.wait_ge(s_dve, (r + 1) * _NBX)
                scr_dst = bass.AP(
                    tensor=scr,
                    offset=r * _SLAB,
                    ap=[[_NCOLS, 128], [_BLK, _NBX], [1, _NCOLS]],
                )
                sync.dma_start(scr_dst, stage[r % 2][:, :]).then_inc(s_st[r % 2], 16)
                if r + 2 < _NBY:
                    _load_f1(sync, r + 2)
                if r > 0:
                    _gathers(r - 1)
                    _store_out(r - 1)
            _gathers(_NBY - 1)
            _store_out(_NBY - 1)
            sync.wait_ge(s_out[0], (_NBY // 2) * 16)
            sync.wait_ge(s_out[1], (_NBY // 2) * 16)

        @blk.scalar
        def _(scalar):
            def _select(r):
                scalar.wait_ge(s_g[r % 2], (r // 2 + 1) * _BY * 16)
                if r >= 2:  # WAR: out store of r-2 read ob[r%2]
                    scalar.wait_ge(s_out[r % 2], (r // 2) * 16)
                src = bass.AP(
                    tensor=gb[r % 2],
                    offset=0,
                    ap=[[_RUN, 128], [_NCOLS + 1, _NBX], [_NB, _K], [1, _K]],
                )
                dst = bass.AP(
                    tensor=ob[r % 2],
                    offset=0,
                    ap=[[_NBX * _ND, 128], [_ND, _NBX], [_K, _K], [1, _K]],
                )
                scalar.activation(
                    dst, src, mybir.ActivationFunctionType.Copy
                ).then_inc(s_sel, 1)

            scalar.wait_ge(s_f2, 16)
            for r in range(_NBY):
                # WAR: matmuls of r-2 read f2row[r%2]
                if r >= 2:
                    scalar.wait_ge(s_pe, (r - 1) * _NBX)
                src2 = bass.AP(
                    tensor=f2p,
                    offset=r * _BY * _WP,
                    ap=[[_HP * _WP, _C], [_BX, _NBX], [_WP, _NA], [1, _NB]],
                )
                scalar.activation(
                    f2row[r % 2][:, :], src2, mybir.ActivationFunctionType.Copy
                ).then_inc(s_act, 1)
                if r > 0:
                    _select(r - 1)
            _select(_NBY - 1)

        @blk.tensor
        def _(tensor):
            for r in range(_NBY):
                tensor.wait_ge(s_f1[r % 2], (r // 2 + 1) * 16)
                tensor.wait_ge(s_act, r + 1)
                for x0 in range(_NBX):
                    n = r * _NBX + x0
                    if n >= _NPS:  # WAR: stage copy freed this psum bank
                        tensor.wait_ge(s_dve, n - _NPS + 1)
                    lhsT = f1r[r % 2][:, x0 * 128 : (x0 + 1) * 128]
                    rhs = f2row[r % 2][:, x0 * _NCOLS : (x0 + 1) * _NCOLS]
                    nc.tensor.matmul(
                        psum[n % _NPS][:, :], lhsT, rhs, start=True, stop=True
                    ).then_inc(s_pe, 1)

        @blk.vector
        def _(vector):
            for r in range(_NBY):
                # WAR: scr store of r-2 read this stage buffer
                if r >= 2:
                    vector.wait_ge(s_st[r % 2], (r // 2) * 16)
                for x0 in range(_NBX):
                    n = r * _NBX + x0
                    vector.wait_ge(s_pe, n + 1)
                    st = stage[r % 2][:, x0 * _NCOLS : (x0 + 1) * _NCOLS]
                    nc.vector.tensor_copy(st, psum[n % _NPS][:, :]).then_inc(s_dve, 1)

    return nc


def _pack_f1(f1_core: np.ndarray) -> np.ndarray:
    """[c, h, w] fp32 -> [c, y0, x0*128 + ry*16 + rx] bf16, pre-scaled 1/c."""
    import ml_dtypes

    v = (f1_core * (1.0 / _C)).reshape(_C, _NBY, _BY, _NBX, _BX)
    v = v.transpose(0, 1, 3, 2, 4)  # c, y0, x0, ry, rx
    return np.ascontiguousarray(v.reshape(_C, _NBY, _NBX * 128)).astype(
        ml_dtypes.bfloat16
    )


def make_in_maps(feature1: np.ndarray, feature2: np.ndarray) -> list:
    import ml_dtypes

    f1 = np.ascontiguousarray(np.asarray(feature1), dtype=np.float32)
    f2 = np.ascontiguousarray(np.asarray(feature2), dtype=np.float32)
    f2p = np.zeros((len(f2), _C, _HP, _WP), dtype=np.float32)
    f2p[:, :, 4 : 4 + _H, 4 : 4 + _W] = f2
    f2pb = f2p.astype(ml_dtypes.bfloat16)
    return [{"f1": _pack_f1(f1[i]), "f2": f2pb[i]} for i in range(len(f1))]


def kernel(feature1: np.ndarray, feature2: np.ndarray) -> np.ndarray:
    from concourse.bass_utils import run_bass_kernel_spmd

    if "nc" not in _CACHE:
        _CACHE["nc"] = _build_nc()
    nc = _CACHE["nc"]

    in_maps = make_in_maps(feature1, feature2)
    res = run_bass_kernel_spmd(nc, in_maps, core_ids=list(range(_B)))
    # [y0][(ry*256+x)*81+d] bf16 -> [d, y, x] fp32
    outs = []
    for i in range(_B):
        o = res.results[i]["out"].astype(np.float32)
        o = o.reshape(_H, _W, _ND).transpose(2, 0, 1)
        outs.append(o)
    return np.ascontiguousarray(np.stack(outs, axis=0))
